# revision 4
# baseline (speedup 1.0000x reference)
"""HGNN encoder (2-layer hypergraph conv) Bass kernel for 8 Trainium2 NeuronCores.

Strategy (1D nnz-parallel over hyperedge columns):
  - Host: compute degree scalings, fold them into per-nnz weights; partition the
    1M nonzeros across 8 cores by hyperedge (col) ranges; build a static padded
    schedule of 128-nnz K-tiles grouped by 128-wide destination blocks.
  - Device, per layer:
      P1: Ye[e] = sum_i w1_i * X[row_i]   (dest = local col blocks, own range)
          gather X rows (dma_gather, bf16) -> build one-hot weight matrix M per
          K-tile on VectorE (iota is_equal dest, * w) -> TensorE matmul
          accumulates each dest block in PSUM -> Ye kept in SBUF, spilled bf16.
      P2: Yn[n] += sum_i w2_i * Ye[col_i] (dest = global row blocks)
          same machinery; partial Yn streamed to DRAM bf16.
      ReduceScatter(partial Yn) -> each core owns an N/8 slice ->
          X@W^T via DMA-transpose load + TensorE (+ReLU for layer 0) ->
          AllGather slice -> X1 table for layer 1.
  - Layer-0 P1 skips the runtime gather: host pre-gathers X0[rows] into the
    K-tile layout (X is replicated input; this is the edge-parallel sharding).
  - Output: each core returns its final N/8 slice; host concatenates.
"""

import numpy as np
import ml_dtypes

from concourse import bass, mybir, bacc, tile, library_config
from concourse.bass_utils import run_bass_kernel_spmd

BF16 = ml_dtypes.bfloat16
P = 128
D = 128
NCORES = 8

FULL_CFG = dict(
    N=50000, E=100000, NNZ=1000000,
    HALF=32768,          # gather-table split point (int16 index limit)
    GCHUNK=8,            # K-tiles per dma_gather call (8*128 = 1024 idxs;
                         # 2048+ idx calls hang the device)
)


# ----------------------------------------------------------------------------
# Host-side preprocessing
# ----------------------------------------------------------------------------

def _wrap_idx_chunks(slots_idx, nch, gchunk):
    """[T*128] int idx array -> [128, nch*gchunk*8] int16 in per-call wrapped
    16-partition layout replicated 8x (dma_gather idx format)."""
    call = gchunk * P           # idxs per call
    scol = call // 16           # idx columns per call
    g = slots_idx.reshape(nch, call)
    a = g.reshape(nch, scol, 16).transpose(0, 2, 1)      # [nch,16,scol]
    a = np.tile(a, (1, 8, 1))                            # [nch,128,scol]
    return np.ascontiguousarray(
        a.transpose(1, 0, 2).reshape(P, nch * scol)).astype(np.int16)


def _slot_major(slots_vec, T):
    """[T*128] -> [128, T] (partition-major for per-partition scalar APs)."""
    return np.ascontiguousarray(slots_vec.reshape(T, P).T)


def preprocess(rows, cols, vals, X0, W0, W1, cfg):
    N, E, NNZ = cfg["N"], cfg["E"], cfg["NNZ"]
    HALF, GCHUNK = cfg["HALF"], cfg["GCHUNK"]
    rows = np.asarray(rows, np.int64)
    cols = np.asarray(cols, np.int64)
    vals = np.asarray(vals, np.float32)
    X0 = np.asarray(X0, np.float32)

    NBLK2 = -(-N // P)
    # make row-block count divisible by 8 so ReduceScatter slices are block-aligned
    NBLK2 = -(-NBLK2 // NCORES) * NCORES
    N_PAD = NBLK2 * P
    SLICE = N_PAD // NCORES

    # degrees -> folded per-nnz weights
    d_e = np.clip(np.bincount(cols, vals, minlength=E), 1e-6, None)
    d_v = np.clip(np.bincount(rows, vals, minlength=N), 1e-6, None)
    dvis = d_v ** -0.5
    de_inv = 1.0 / d_e
    w1 = (vals * dvis[rows] * de_inv[cols]).astype(np.float32)
    w2 = (vals * dvis[rows]).astype(np.float32)

    # column ranges balanced by nnz count
    ccnt = np.bincount(cols, minlength=E)
    csum = np.cumsum(ccnt)
    marks = [np.searchsorted(csum, NNZ * (c + 1) / NCORES) for c in range(NCORES - 1)]
    bounds = np.array([0] + [int(m) + 1 for m in marks] + [E])
    widths = bounds[1:] - bounds[:-1]
    E_LOC = int(-(-widths.max() // P) * P)
    NB1 = E_LOC // P
    col_core = np.searchsorted(bounds[1:], cols, side="right")

    # per-core raw shard data
    per = []
    for c in range(NCORES):
        m = col_core == c
        per.append(dict(
            r=rows[m], k=(cols[m] - bounds[c]),
            w1=w1[m], w2=w2[m],
        ))

    # ---- pass-1 schedule: groups (h, b) = (row-half, local col block) ----
    cnt1 = np.zeros((NCORES, 2 * NB1), np.int64)
    for c in range(NCORES):
        h = (per[c]["r"] >= HALF).astype(np.int64)
        g = h * NB1 + per[c]["k"] // P
        cnt1[c] = np.bincount(g, minlength=2 * NB1)
    tiles1 = -(-cnt1.max(axis=0) // P)                  # [2*NB1]
    # pad each phase to a multiple of GCHUNK (pad tiles attach to no block)
    ph_tiles = [int(tiles1[:NB1].sum()), int(tiles1[NB1:].sum())]
    ph_tiles_pad = [max(-(-t // GCHUNK) * GCHUNK, GCHUNK) for t in ph_tiles]
    PH1_SPLIT = ph_tiles_pad[0]
    T1 = sum(ph_tiles_pad)
    nch1 = T1 // GCHUNK
    # slot base (in tiles) of each (h,b) group
    base1 = np.zeros(2 * NB1, np.int64)
    acc = 0
    for h in range(2):
        for b in range(NB1):
            base1[h * NB1 + b] = acc
            acc += tiles1[h * NB1 + b]
        acc = sum(ph_tiles_pad[:h + 1])

    # ---- pass-2 schedule: groups = global row blocks ----
    cnt2 = np.zeros((NCORES, NBLK2), np.int64)
    for c in range(NCORES):
        cnt2[c] = np.bincount(per[c]["r"] // P, minlength=NBLK2)
    tiles2 = -(-cnt2.max(axis=0) // P)
    T2_real = int(tiles2.sum())
    T2 = max(-(-T2_real // GCHUNK) * GCHUNK, GCHUNK)
    nch2 = T2 // GCHUNK
    base2 = np.concatenate([[0], np.cumsum(tiles2)[:-1]])

    X0b = X0.astype(BF16)
    W0T = np.ascontiguousarray(np.asarray(W0, np.float32).T).astype(BF16)
    W1T = np.ascontiguousarray(np.asarray(W1, np.float32).T).astype(BF16)

    in_maps = []
    for c in range(NCORES):
        pc = per[c]
        r, k = pc["r"], pc["k"]
        h = (r >= HALF).astype(np.int64)

        # pass 1 slot assignment
        g = h * NB1 + k // P
        o1 = np.lexsort((k, h))
        gs = g[o1]
        # rank within group
        grp_start = np.zeros(len(gs), np.int64)
        if len(gs):
            newg = np.ones(len(gs), bool)
            newg[1:] = gs[1:] != gs[:-1]
            starts = np.nonzero(newg)[0]
            grp_start = starts[np.cumsum(newg) - 1]
        rank = np.arange(len(gs)) - grp_start
        s1 = base1[gs] * P + rank

        m1d = np.zeros(T1 * P, np.float32)
        m1w = np.zeros(T1 * P, np.float32)
        gi1 = np.zeros(T1 * P, np.int64)
        m1d[s1] = (k[o1] % P).astype(np.float32)
        m1w[s1] = pc["w1"][o1]
        gi1[s1] = r[o1] - h[o1] * HALF
        xg = np.zeros((T1 * P, D), BF16)
        xg[s1] = X0b[r[o1]]
        # chunk-partition-major layout for dense L0P1 loads
        xg_pm = np.ascontiguousarray(
            xg.reshape(nch1, GCHUNK, P, D).transpose(0, 2, 1, 3)
        ).reshape(nch1 * P, GCHUNK * D)

        # pass 2 slot assignment
        g2 = r // P
        o2 = np.argsort(g2, kind="stable")
        gs2 = g2[o2]
        grp_start2 = np.zeros(len(gs2), np.int64)
        if len(gs2):
            newg = np.ones(len(gs2), bool)
            newg[1:] = gs2[1:] != gs2[:-1]
            starts = np.nonzero(newg)[0]
            grp_start2 = starts[np.cumsum(newg) - 1]
        rank2 = np.arange(len(gs2)) - grp_start2
        s2 = base2[gs2] * P + rank2

        m2d = np.zeros(T2 * P, np.float32)
        m2w = np.zeros(T2 * P, np.float32)
        gi2 = np.zeros(T2 * P, np.int64)
        m2d[s2] = (r[o2] % P).astype(np.float32)
        m2w[s2] = pc["w2"][o2]
        gi2[s2] = k[o2]

        in_maps.append({
            "xg": xg_pm,
            "gidx1": _wrap_idx_chunks(gi1, nch1, GCHUNK),
            "m1d": _slot_major(m1d, T1), "m1w": _slot_major(m1w, T1),
            "gidx2": _wrap_idx_chunks(gi2, nch2, GCHUNK),
            "m2d": _slot_major(m2d, T2), "m2w": _slot_major(m2w, T2),
            "w0t": W0T, "w1t": W1T,
        })

    sched = dict(
        NB1=NB1, NBLK2=NBLK2, E_LOC=E_LOC, N_PAD=N_PAD, SLICE=SLICE,
        T1=T1, T2=T2, nch1=nch1, nch2=nch2,
        tiles1=tiles1.reshape(2, NB1), base1=base1.reshape(2, NB1),
        tiles2=tiles2, base2=base2,
        HALF=cfg["HALF"], GCHUNK=GCHUNK, PH1_SPLIT=PH1_SPLIT,
    )
    return in_maps, sched


# ----------------------------------------------------------------------------
# Device kernel builder
# ----------------------------------------------------------------------------

def build_kernel(nc, tc, aps, sched):
    NB1, NBLK2 = sched["NB1"], sched["NBLK2"]
    E_LOC, N_PAD, SLICE = sched["E_LOC"], sched["N_PAD"], sched["SLICE"]
    T1, T2 = sched["T1"], sched["T2"]
    GC = sched["GCHUNK"]
    HALF = sched["HALF"]
    tiles1, base1 = sched["tiles1"], sched["base1"]
    tiles2, base2 = sched["tiles2"], sched["base2"]
    MCHUNK = 512  # meta tiles per load

    nc.gpsimd.load_library(library_config.mlp)
    rg = [list(range(NCORES))]

    from contextlib import ExitStack
    ctx = ExitStack()
    sbuf = ctx.enter_context(tc.tile_pool(name="sbuf", bufs=1))
    gpool = ctx.enter_context(tc.tile_pool(name="gpool", bufs=3))
    mpool = ctx.enter_context(tc.tile_pool(name="mpool", bufs=2))
    ipool = ctx.enter_context(tc.tile_pool(name="ipool", bufs=3))
    wpool = ctx.enter_context(tc.tile_pool(name="wpool", bufs=4))
    psum = ctx.enter_context(tc.tile_pool(name="psum", bufs=6, space="PSUM"))
    bpsum = ctx.enter_context(tc.tile_pool(name="bpsum", bufs=2, space="PSUM"))
    dram = ctx.enter_context(tc.tile_pool(name="dram", bufs=1, space="DRAM"))

    # constants
    iota_i = sbuf.tile([P, P], mybir.dt.int16)
    nc.gpsimd.iota(iota_i[:], pattern=[[1, P]], base=0, channel_multiplier=0)
    iota_b = sbuf.tile([P, P], mybir.dt.bfloat16)
    nc.vector.tensor_copy(iota_b[:], iota_i[:])
    w0t_sb = sbuf.tile([P, P], mybir.dt.bfloat16)
    w1t_sb = sbuf.tile([P, P], mybir.dt.bfloat16)
    nc.sync.dma_start(out=w0t_sb[:], in_=aps["w0t"][:])
    nc.sync.dma_start(out=w1t_sb[:], in_=aps["w1t"][:])

    # persistent SBUF: Ye accumulator (fp32) for pass 1
    ye_acc = sbuf.tile([P, NB1 * D], mybir.dt.float32)

    # internal DRAM
    ye = dram.tile([E_LOC, D], mybir.dt.bfloat16)
    ynp = dram.tile([N_PAD, D], mybir.dt.bfloat16)
    ysl = dram.tile([SLICE, D], mybir.dt.bfloat16)
    x1loc = dram.tile([SLICE, D], mybir.dt.bfloat16)
    x1 = dram.tile([N_PAD, D], mybir.dt.bfloat16, addr_space="Shared")

    def seg_pass(layer, pass_no):
        """Emit one segmented-matmul pass. Returns nothing; writes ye or ynp."""
        if pass_no == 1:
            nch, T, tiles, base = sched["nch1"], T1, tiles1, base1
            md_ap, mw_ap, gi_ap = aps["m1d"], aps["m1w"], aps["gidx1"]
            phases = 2
        else:
            nch, T, tiles, base = sched["nch2"], T2, tiles2, base2
            md_ap, mw_ap, gi_ap = aps["m2d"], aps["m2w"], aps["gidx2"]
            phases = 1

        gstate = {"ci": -1, "tile": None}
        mstate = {"ci": -1, "d": None, "w": None}
        scol = GC * P // 16

        def ensure_chunk(t):
            ci = t // GC
            if gstate["ci"] != ci:
                gb = gpool.tile([P, GC, D], mybir.dt.bfloat16, tag="gb")
                if pass_no == 1 and layer == 0:
                    nc.sync.dma_start(
                        out=gb[:],
                        in_=aps["xg"][ci * P:(ci + 1) * P, :])
                else:
                    it = ipool.tile([P, scol], mybir.dt.int16, tag="it")
                    nc.sync.dma_start(
                        out=it[:], in_=gi_ap[:, ci * scol:(ci + 1) * scol])
                    if pass_no == 1:
                        # phase decides which half-table; chunks never span phases
                        ph = 0 if (ci * GC) < sched_ph1_split else 1
                        tbl = x1[0:HALF, :] if ph == 0 else x1[HALF:N_PAD, :]
                    else:
                        tbl = ye[:]
                    nc.gpsimd.dma_gather(
                        out_ap=gb[:], in_ap=tbl, idxs_ap=it[:],
                        num_idxs=GC * P, num_idxs_reg=GC * P, elem_size=D)
                gstate["ci"] = ci
                gstate["tile"] = gb
            return gstate["tile"]

        def ensure_meta(t):
            ci = t // MCHUNK
            if mstate["ci"] != ci:
                n = min(MCHUNK, T - ci * MCHUNK)
                dt_ = mpool.tile([P, n], mybir.dt.float32, tag=f"md{pass_no}")
                wt_ = mpool.tile([P, n], mybir.dt.float32, tag=f"mw{pass_no}")
                nc.sync.dma_start(out=dt_[:], in_=md_ap[:, ci * MCHUNK:ci * MCHUNK + n])
                nc.sync.dma_start(out=wt_[:], in_=mw_ap[:, ci * MCHUNK:ci * MCHUNK + n])
                mstate.update(ci=ci, d=dt_, w=wt_)
            return mstate["d"], mstate["w"], t - ci * MCHUNK

        sched_ph1_split = sched["PH1_SPLIT"]

        nblocks = NB1 if pass_no == 1 else NBLK2
        stage2 = None
        for hb in range(phases * nblocks):
            h, b = divmod(hb, nblocks)
            ntile = int(tiles[h][b] if pass_no == 1 else tiles[b])
            pt = None
            if ntile > 0:
                pt = psum.tile([P, D], mybir.dt.float32, space="PSUM",
                               tag="ps")
                for j in range(ntile):
                    t = int((base[h][b] if pass_no == 1 else base[b]) + j)
                    gb = ensure_chunk(t)
                    mdt, mwt, tloc = ensure_meta(t)
                    m = wpool.tile([P, P], mybir.dt.bfloat16, tag="mtile")
                    nc.vector.tensor_scalar(
                        out=m[:], in0=iota_b[:],
                        scalar1=mdt[:, tloc:tloc + 1],
                        scalar2=mwt[:, tloc:tloc + 1],
                        op0=mybir.AluOpType.is_equal,
                        op1=mybir.AluOpType.mult)
                    nc.tensor.matmul(
                        out=pt[:], lhsT=m[:], rhs=gb[:, t % GC, :],
                        start=(j == 0), stop=(j == ntile - 1))

            if pass_no == 1:
                dst = ye_acc[:, b * D:(b + 1) * D]
                if h == 0:
                    if ntile > 0:
                        nc.scalar.activation(
                            out=dst, in_=pt[:],
                            func=mybir.ActivationFunctionType.Copy)
                    else:
                        nc.vector.memset(dst, 0)
                else:
                    if ntile > 0:
                        nc.vector.tensor_tensor(
                            out=dst, in0=pt[:], in1=dst,
                            op=mybir.AluOpType.add)
                    # spill finished block to DRAM as bf16 gather table
                    yb = wpool.tile([P, D], mybir.dt.bfloat16, tag="yspill")
                    nc.scalar.activation(
                        out=yb[:], in_=dst,
                        func=mybir.ActivationFunctionType.Copy)
                    nc.sync.dma_start(out=ye[b * P:(b + 1) * P, :], in_=yb[:])
            else:
                yb = wpool.tile([P, D], mybir.dt.bfloat16, tag="ynspill")
                if ntile > 0:
                    nc.scalar.activation(
                        out=yb[:], in_=pt[:],
                        func=mybir.ActivationFunctionType.Copy)
                else:
                    nc.vector.memset(yb[:], 0)
                nc.sync.dma_start(out=ynp[b * P:(b + 1) * P, :], in_=yb[:])

    def boundary(layer):
        """ReduceScatter partial Yn, apply W (+relu for layer0), produce x1/out."""
        nc.gpsimd.collective_compute(
            "ReduceScatter", mybir.AluOpType.add, replica_groups=rg,
            ins=[ynp[:]], outs=[ysl[:]])
        yt = sbuf.tile([P, SLICE], mybir.dt.bfloat16, name=f"yt{layer}")
        nc.sync.dma_start_transpose(yt[:], ysl[:])
        nblk = SLICE // P
        for nb in range(nblk):
            pz = bpsum.tile([P, D], mybir.dt.float32, space="PSUM", tag="bps")
            nc.tensor.matmul(
                out=pz[:], lhsT=yt[:, nb * P:(nb + 1) * P],
                rhs=(w0t_sb[:] if layer == 0 else w1t_sb[:]),
                start=True, stop=True)
            if layer == 0:
                xb = wpool.tile([P, D], mybir.dt.bfloat16, tag="xspill")
                nc.scalar.activation(
                    out=xb[:], in_=pz[:],
                    func=mybir.ActivationFunctionType.Relu)
                nc.sync.dma_start(out=x1loc[nb * P:(nb + 1) * P, :], in_=xb[:])
            else:
                ob = wpool.tile([P, D], mybir.dt.float32, tag="ospill")
                nc.vector.tensor_copy(ob[:], pz[:])
                nc.sync.dma_start(out=aps["out"][nb * P:(nb + 1) * P, :], in_=ob[:])
        if layer == 0:
            nc.gpsimd.collective_compute(
                "AllGather", mybir.AluOpType.bypass, replica_groups=rg,
                ins=[x1loc[:]], outs=[x1[:]])

    for layer in range(2):
        seg_pass(layer, 1)
        seg_pass(layer, 2)
        boundary(layer)
    ctx.close()


# ----------------------------------------------------------------------------
# Top-level entry
# ----------------------------------------------------------------------------

def make_nc(in_map0, sched):
    nc = bacc.Bacc("TRN2", target_bir_lowering=False, debug=False,
                   num_devices=NCORES)
    aps = {}
    for k, v in in_map0.items():
        aps[k] = nc.dram_tensor(k, list(v.shape), mybir.dt.from_np(v.dtype),
                                kind="ExternalInput").ap()
    aps["out"] = nc.dram_tensor("out", [sched["SLICE"], D], mybir.dt.float32,
                                kind="ExternalOutput").ap()
    with tile.TileContext(nc) as tc:
        build_kernel(nc, tc, aps, sched)
    nc.compile()
    return nc


def prepare(inputs, cfg):
    in_maps, sched = preprocess(
        inputs["rows"], inputs["cols"], inputs["vals"],
        inputs["X0"], inputs["W0"], inputs["W1"], cfg)
    nc = make_nc(in_maps[0], sched)
    return nc, in_maps, sched


def run(inputs, cfg, trace=False):
    nc, in_maps, sched = prepare(inputs, cfg)
    res = run_bass_kernel_spmd(nc, in_maps, core_ids=list(range(NCORES)),
                               trace=trace)
    outs = [res.results[c]["out"] for c in range(NCORES)]
    Z = np.concatenate(outs, axis=0)[:cfg["N"]]
    return Z.astype(np.float32), res


def kernel(rows, cols, vals, X0, W0, W1, num_hpo, num_edges):
    cfg = dict(FULL_CFG)
    cfg["N"] = int(num_hpo)
    cfg["E"] = int(num_edges)
    cfg["NNZ"] = int(len(np.asarray(rows)))
    Z, _ = run(dict(rows=rows, cols=cols, vals=vals, X0=X0, W0=W0, W1=W1), cfg)
    return Z
